# revision 1
# baseline (speedup 1.0000x reference)
"""MultiHeadAttention (B=4,T=2048,D=2048,NQ=16,NK=8,H=128) on 8 trn2 cores.

Sharding: core c -> batch b=c//2, half=c%2. Each core computes the partial
output for batch b restricted to q-heads [half*8, half*8+8) (kv-heads
[half*4, half*4+4)); host sums the two partials per batch (o_proj
contraction over heads is split across the core pair).
"""
import numpy as np
import concourse.bass as bass
import concourse.tile as tile
from concourse import bacc, mybir
from concourse import bass_utils

B, T, D = 4, 2048, 2048
NQ, NK, H = 16, 8, 128
NH, NKV = 8, 4          # per-core q heads / kv heads
THETA = 10000.0
EPS = 1e-6
TCH = 512               # projection-pass T chunk
NCH = T // TCH
NDK = D // 128
QCH = 512               # attention q chunk
NQC = T // QCH
NTB = T // 128

f32 = mybir.dt.float32
f32r = mybir.dt.float32r
AF = mybir.ActivationFunctionType

TRACE = False
LAST_EXEC_NS = None
_CACHE = {}


def _install_hook():
    import contextlib, ctypes, sys, types
    if "antenv.axon_hooks" in sys.modules:
        return
    lib = ctypes.CDLL("/opt/axon/libaxon_pjrt.so")
    lib.axon_start_nrt_profile.argtypes = [ctypes.POINTER(ctypes.c_int64), ctypes.c_size_t]
    lib.axon_start_nrt_profile.restype = ctypes.c_int64
    lib.axon_stop_nrt_profile.argtypes = [ctypes.c_char_p]
    lib.axon_stop_nrt_profile.restype = ctypes.c_int64

    @contextlib.contextmanager
    def _hook(output_dir, device_ids):
        import jax
        jax.devices()
        ids = (ctypes.c_int64 * len(device_ids))(*device_ids) if device_ids else None
        rc = lib.axon_start_nrt_profile(ids, len(device_ids) if device_ids else 0)
        if rc != 0:
            raise RuntimeError(f"axon_start_nrt_profile rc={rc}")
        try:
            yield
        finally:
            n = lib.axon_stop_nrt_profile(str(output_dir).encode())
            if n < 0:
                raise RuntimeError(f"axon_stop_nrt_profile rc={n}")

    mod = types.ModuleType("antenv.axon_hooks")
    mod.get_axon_ntff_profile_hook = lambda: _hook
    mod.set_axon_ntff_profile_hook = lambda h: None
    sys.modules["antenv.axon_hooks"] = mod
    bass_utils.upload_artifacts = lambda tmpdir: "local://" + str(tmpdir)


def _build():
    nc = bacc.Bacc("TRN2", target_bir_lowering=False, debug=False, num_devices=8)
    xt_ap = nc.dram_tensor("xt", [D, T], f32r, kind="ExternalInput").ap()
    wq_ap = nc.dram_tensor("wq", [128, NH * NDK * 128], f32r, kind="ExternalInput").ap()
    wk_ap = nc.dram_tensor("wk", [128, NKV * NDK * 128], f32r, kind="ExternalInput").ap()
    wv_ap = nc.dram_tensor("wv", [128, NKV * NDK * 128], f32r, kind="ExternalInput").ap()
    wo_ap = nc.dram_tensor("wo", [128, NH * D], f32r, kind="ExternalInput").ap()
    cs_ap = nc.dram_tensor("cs", [128, T], f32, kind="ExternalInput").ap()
    sn_ap = nc.dram_tensor("sn", [128, T], f32, kind="ExternalInput").ap()
    qsc_ap = nc.dram_tensor("qsc", [128, 1], f32, kind="ExternalInput").ap()
    ksc_ap = nc.dram_tensor("ksc", [128, 1], f32, kind="ExternalInput").ap()
    cm_ap = nc.dram_tensor("cm", [128, 4 * QCH], f32r, kind="ExternalInput").ap()
    id_ap = nc.dram_tensor("ident", [128, 128], f32r, kind="ExternalInput").ap()
    oc_ap = nc.dram_tensor("onesc", [128, 1], f32r, kind="ExternalInput").ap()
    out_ap = nc.dram_tensor("out", [T, D], f32, kind="ExternalOutput").ap()

    with tile.TileContext(nc) as tc:
        with tc.tile_pool(name="perm", bufs=1) as perm, \
             tc.tile_pool(name="psum", bufs=1, space="PSUM") as pp, \
             tc.tile_pool(name="dram", bufs=1, space="DRAM") as dpool:
            ident = perm.tile([128, 128], f32r)
            nc.sync.dma_start(ident[:], id_ap[:])
            ones_col = perm.tile([128, 1], f32r)
            nc.sync.dma_start(ones_col[:], oc_ap[:])
            ones_row = perm.tile([1, 128], f32)
            nc.vector.memset(ones_row[:], 1.0)
            qsc_t = perm.tile([128, 1], f32)
            nc.sync.dma_start(qsc_t[:], qsc_ap[:])
            ksc_t = perm.tile([128, 1], f32)
            nc.sync.dma_start(ksc_t[:], ksc_ap[:])
            cm_t = perm.tile([128, 4 * QCH], f32r)
            nc.sync.dma_start(cm_t[:], cm_ap[:])
            eps_t = perm.tile([1, 1], f32)
            nc.vector.memset(eps_t[:], EPS)
            qt_sp = dpool.tile([128, NH * T], f32r)

            def build_xts(pool, ch):
                t0 = ch * TCH
                xts = pool.tile([128, NDK * TCH], f32r, tag="xts")
                for dk in range(NDK):
                    nc.sync.dma_start(xts[:, dk * TCH:(dk + 1) * TCH],
                                      xt_ap[dk * 128:(dk + 1) * 128, t0:t0 + TCH])
                cs_c = pool.tile([128, TCH], f32, tag="csc")
                nc.sync.dma_start(cs_c[:], cs_ap[:, t0:t0 + TCH])
                sn_c = pool.tile([128, TCH], f32, tag="snc")
                nc.sync.dma_start(sn_c[:], sn_ap[:, t0:t0 + TCH])
                return xts, cs_c, sn_c

            MUL = mybir.AluOpType.mult

            def drain_group(pool, accs, sc_t, cs_c, sn_c, dsts):
                """accs: 4 psum (128,TCH) f32 -> RMSNorm*(gain) + RoPE -> dsts f32r.

                Ln/Exp batched per group so the ACT table loads amortize."""
                n = len(accs)
                tmp4 = pool.tile([128, 4 * TCH], f32, tag="dtmp4")
                lrs = []
                for i in range(n):
                    tmp = tmp4[:, i * TCH:(i + 1) * TCH]
                    nc.vector.tensor_copy(tmp, accs[i])
                    sq = pool.tile([128, TCH], f32r, tag="dsq", name=f"sq{i}")
                    nc.vector.tensor_mul(sq[:], tmp, tmp)
                    row = pp.tile([1, 512], f32, tag="row", name=f"row{i}")
                    nc.tensor.matmul(row[:, 0:TCH], ones_col[:], sq[:], start=True, stop=True)
                    lr = pool.tile([1, TCH], f32, tag="dlr", bufs=4, name=f"lr{i}")
                    nc.scalar.activation(lr[:], row[:, 0:TCH], AF.Ln, bias=eps_t[:], scale=1.0 / H)
                    lrs.append(lr)
                for i in range(n):
                    rstd = pool.tile([1, TCH], f32, tag="drstd", bufs=2, name=f"rstd{i}")
                    nc.scalar.activation(rstd[:], lrs[i][:], AF.Exp, scale=-0.5)
                    bc = pp.tile([128, 512], f32, tag="bc", name=f"bc{i}")
                    nc.tensor.matmul(bc[:, 0:TCH], ones_row[:], rstd[:], start=True, stop=True)
                    tmp = tmp4[:, i * TCH:(i + 1) * TCH]
                    qn = pool.tile([128, TCH], f32, tag="dqn", name=f"qn{i}")
                    nc.vector.scalar_tensor_tensor(qn[:], tmp, sc_t[:], bc[:, 0:TCH], op0=MUL, op1=MUL)
                    qsw = pool.tile([128, TCH], f32, tag="dqsw", bufs=2, name=f"qsw{i}")
                    nc.sync.dma_start(qsw[0:64, :], qn[64:128, :])
                    nc.sync.dma_start(qsw[64:128, :], qn[0:64, :])
                    ta = pool.tile([128, TCH], f32, tag="dta", name=f"ta{i}")
                    nc.vector.tensor_mul(ta[:], qn[:], cs_c[:])
                    tb = pool.tile([128, TCH], f32, tag="dtb", name=f"tb{i}")
                    nc.vector.tensor_mul(tb[:], qsw[:], sn_c[:])
                    nc.vector.tensor_add(dsts[i], ta[:], tb[:])

            # ---- pass A: q projection -> norm/rope -> DRAM spill ----
            with tc.tile_pool(name="pa", bufs=1) as pa:
                wq_t = pa.tile([128, NH * NDK * 128], f32r)
                nc.sync.dma_start(wq_t[:], wq_ap[:])
                for ch in range(NCH):
                    t0 = ch * TCH
                    xts, cs_c, sn_c = build_xts(pa, ch)
                    for g in range(2):
                        accs = [pp.tile([128, 512], f32, tag=f"acc{i}", name=f"acc{i}") for i in range(4)]
                        for dk in range(NDK):
                            for hh in range(4):
                                h = g * 4 + hh
                                nc.tensor.matmul(
                                    accs[hh][:, 0:TCH],
                                    wq_t[:, (h * NDK + dk) * 128:(h * NDK + dk + 1) * 128],
                                    xts[:, dk * TCH:(dk + 1) * TCH],
                                    start=(dk == 0), stop=(dk == NDK - 1))
                        stgs = [pa.tile([128, TCH], f32r, tag="stg", bufs=8, name=f"stg{i}")
                                for i in range(4)]
                        drain_group(pa, [a[:, 0:TCH] for a in accs], qsc_t, cs_c, sn_c,
                                    [s[:] for s in stgs])
                        for hh in range(4):
                            h = g * 4 + hh
                            nc.sync.dma_start(qt_sp[:, h * T + t0: h * T + t0 + TCH], stgs[hh][:])

            # ---- pass B: k,v projection; k -> kT resident, v -> vT resident ----
            with tc.tile_pool(name="kvp", bufs=1) as kvpool:
                kT = kvpool.tile([128, NKV * T], f32r)
                vT = kvpool.tile([128, NKV * T], f32r)
                with tc.tile_pool(name="pb", bufs=1) as pb:
                    wk_t = pb.tile([128, NKV * NDK * 128], f32r)
                    nc.sync.dma_start(wk_t[:], wk_ap[:])
                    wv_t = pb.tile([128, NKV * NDK * 128], f32r)
                    nc.sync.dma_start(wv_t[:], wv_ap[:])
                    for ch in range(NCH):
                        t0 = ch * TCH
                        xts, cs_c, sn_c = build_xts(pb, ch)
                        # k group
                        accs = [pp.tile([128, 512], f32, tag=f"acc{i}", name=f"acc{i}") for i in range(4)]
                        for dk in range(NDK):
                            for kv in range(NKV):
                                nc.tensor.matmul(
                                    accs[kv][:, 0:TCH],
                                    wk_t[:, (kv * NDK + dk) * 128:(kv * NDK + dk + 1) * 128],
                                    xts[:, dk * TCH:(dk + 1) * TCH],
                                    start=(dk == 0), stop=(dk == NDK - 1))
                        drain_group(pb, [a[:, 0:TCH] for a in accs], ksc_t, cs_c, sn_c,
                                    [kT[:, kv * T + t0: kv * T + t0 + TCH] for kv in range(NKV)])
                        # v group
                        accs = [pp.tile([128, 512], f32, tag=f"acc{i}", name=f"acc{i}") for i in range(4)]
                        for dk in range(NDK):
                            for kv in range(NKV):
                                nc.tensor.matmul(
                                    accs[kv][:, 0:TCH],
                                    wv_t[:, (kv * NDK + dk) * 128:(kv * NDK + dk + 1) * 128],
                                    xts[:, dk * TCH:(dk + 1) * TCH],
                                    start=(dk == 0), stop=(dk == NDK - 1))
                        for kv in range(NKV):
                            vtmp = pb.tile([128, TCH], f32r, tag="vtmp")
                            nc.vector.tensor_copy(vtmp[:], accs[kv][:, 0:TCH])
                            trp = pp.tile([128, 512], f32r, tag="tr")
                            for j in range(TCH // 128):
                                nc.tensor.transpose(
                                    trp[:, j * 128:(j + 1) * 128],
                                    vtmp[:, j * 128:(j + 1) * 128], ident[:])
                            nc.vector.tensor_copy(
                                vT[:, kv * T + t0: kv * T + t0 + TCH], trp[:, 0:TCH])

                # ---- attention: softmax(q k^T) v, causal, no max-subtraction ----
                with tc.tile_pool(name="ap2", bufs=1) as ap2:
                    attn = ap2.tile([128, NH * T], f32r)
                    den_all = ap2.tile([NH * NQC, QCH], f32)
                    with tc.tile_pool(name="at", bufs=1) as at:
                        for h in range(NH):
                            kv = h // 2
                            qh = at.tile([128, T], f32r, tag="qh", bufs=2)
                            nc.sync.dma_start(qh[:], qt_sp[:, h * T:(h + 1) * T])
                            for qi in range(NQC):
                                o_ps = pp.tile([128, 512], f32, tag="acc2")
                                acc_sb = at.tile([128, QCH], f32r, tag="asb")
                                nkj = 4 * qi + 4
                                for kj in range(nkj):
                                    s_ps = pp.tile([128, 512], f32, tag=f"acc{kj % 2 and 1 or 0}")
                                    nc.tensor.matmul(
                                        s_ps[:], kT[:, kv * T + kj * 128: kv * T + (kj + 1) * 128],
                                        qh[:, qi * QCH:(qi + 1) * QCH], start=True, stop=True)
                                    pt = at.tile([128, QCH], f32r, tag="pt", bufs=3)
                                    nc.scalar.activation(pt[:], s_ps[:], AF.Exp)
                                    m = kj - 4 * qi
                                    if m >= 0:
                                        nc.vector.tensor_mul(pt[:], pt[:], cm_t[:, m * QCH:(m + 1) * QCH])
                                    if kj == 0:
                                        nc.vector.tensor_copy(acc_sb[:], pt[:])
                                    else:
                                        nc.vector.tensor_add(acc_sb[:], acc_sb[:], pt[:])
                                    nc.tensor.matmul(
                                        o_ps[:], vT[:, kv * T + kj * 128: kv * T + (kj + 1) * 128],
                                        pt[:], start=(kj == 0), stop=(kj == nkj - 1))
                                row = pp.tile([1, 512], f32, tag="row")
                                nc.tensor.matmul(row[:], ones_col[:], acc_sb[:], start=True, stop=True)
                                dcp = at.tile([1, QCH], f32, tag="dcp", bufs=2)
                                nc.vector.tensor_copy(dcp[:], row[:])
                                p = h * NQC + qi
                                nc.sync.dma_start(den_all[p:p + 1, :], dcp[:])
                                nc.vector.tensor_copy(
                                    attn[:, h * T + qi * QCH: h * T + (qi + 1) * QCH], o_ps[:])
                        # batched softmax denominators: rden = exp(-ln(den))
                        lden = at.tile([NH * NQC, QCH], f32, tag="lden")
                        nc.scalar.activation(lden[:], den_all[:], AF.Ln)
                        rden = at.tile([NH * NQC, QCH], f32, tag="rden")
                        nc.scalar.activation(rden[:], lden[:], AF.Exp, scale=-1.0)
                        for p in range(NH * NQC):
                            rd1 = at.tile([1, QCH], f32, tag="rd1", bufs=2, name=f"rd1_{p}")
                            nc.sync.dma_start(rd1[:], rden[p:p + 1, :])
                            bc = pp.tile([128, 512], f32, tag="bc")
                            nc.tensor.matmul(bc[:], ones_row[:], rd1[:], start=True, stop=True)
                            h, qi = divmod(p, NQC)
                            sl = attn[:, h * T + qi * QCH: h * T + (qi + 1) * QCH]
                            nc.vector.tensor_mul(sl, sl, bc[:])

                    # ---- o_proj partial: out[tc,dc] = sum_h attnT_h^T @ wo_h ----
                    with tc.tile_pool(name="op", bufs=1) as opool:
                        for dc in range(4):
                            wos = []
                            for h in range(NH):
                                w = opool.tile([128, 512], f32r, tag="wo", bufs=8)
                                nc.sync.dma_start(w[:], wo_ap[:, h * D + dc * 512: h * D + (dc + 1) * 512])
                                wos.append(w)
                            for ti in range(NTB):
                                ops = pp.tile([128, 512], f32, tag=f"acc{ti % 2}")
                                for h in range(NH):
                                    nc.tensor.matmul(
                                        ops[:], attn[:, h * T + ti * 128: h * T + (ti + 1) * 128],
                                        wos[h][:], start=(h == 0), stop=(h == NH - 1))
                                stg = opool.tile([128, 512], f32, tag="ostg", bufs=2)
                                nc.vector.tensor_copy(stg[:], ops[:])
                                nc.sync.dma_start(
                                    out_ap[ti * 128:(ti + 1) * 128, dc * 512:(dc + 1) * 512], stg[:])

    nc.compile()
    return nc


def _pack(w):
    """(nh, D, H) -> (128, nh*NDK*128): col block (h*NDK+dk)*128 = w[h, dk*128:+128, :]."""
    nh = w.shape[0]
    a = w.reshape(nh, NDK, 128, H).transpose(2, 0, 1, 3)
    return np.ascontiguousarray(a.reshape(128, nh * NDK * H))


def _numpy_ref(x, mask, position, qp, kvp, op, qns, kns):
    def rms(v, s):
        var = (v * v).mean(-1, keepdims=True)
        return v / np.sqrt(var + EPS) * (1.0 + s)

    def rope(v, pos):
        ts = THETA ** (np.arange(64, dtype=np.float32) * 2.0 / H)
        ang = pos.astype(np.float32)[:, :, None, None] / ts
        sn, cs = np.sin(ang), np.cos(ang)
        x1, x2 = v[..., :64], v[..., 64:]
        return np.concatenate([x1 * cs - x2 * sn, x2 * cs + x1 * sn], -1)

    q = np.einsum('BTD,NDH->BTNH', x, qp)
    k = np.einsum('BTD,KDH->BTKH', x, kvp[0])
    v = np.einsum('BTD,KDH->BTKH', x, kvp[1])
    q = rope(rms(q, qns), position) * (H ** -0.5)
    k = rope(rms(k, kns), position)
    q = q.transpose(0, 2, 1, 3)
    k = np.repeat(k.transpose(0, 2, 1, 3), NQ // NK, 1)
    v = np.repeat(v.transpose(0, 2, 1, 3), NQ // NK, 1)
    s = np.einsum('BHtD,BHTD->BHtT', q, k) / np.sqrt(np.float32(H))
    s = np.where(mask[:, None], s, np.float32(-2.3819763e+38))
    s = s - s.max(-1, keepdims=True)
    w = np.exp(s)
    w /= w.sum(-1, keepdims=True)
    o = np.einsum('BHtT,BHTD->BHtD', w, v)
    return np.einsum('BNTH,NHD->BTD', o, op).astype(np.float32)


def kernel(**inputs):
    global LAST_EXEC_NS
    x = np.asarray(inputs["x"], np.float32)
    mask = np.asarray(inputs["mask"])
    position = np.asarray(inputs["position"])
    qp = np.asarray(inputs["q_proj"], np.float32)
    kvp = np.asarray(inputs["kv_proj"], np.float32)
    op = np.asarray(inputs["o_proj"], np.float32)
    qns = np.asarray(inputs["q_norm_scale"], np.float32)
    kns = np.asarray(inputs["k_norm_scale"], np.float32)

    tril = np.tril(np.ones((T, T), bool))
    if mask.shape != (B, T, T) or not all(np.array_equal(mask[b], tril) for b in range(B)):
        return _numpy_ref(x, mask, position, qp, kvp, op, qns, kns)

    if "nc" not in _CACHE:
        _CACHE["nc"] = _build()
    nc = _CACHE["nc"]

    halves = []
    for half in range(2):
        halves.append((
            _pack(qp[half * NH:(half + 1) * NH]),
            _pack(kvp[0, half * NKV:(half + 1) * NKV]),
            _pack(kvp[1, half * NKV:(half + 1) * NKV]),
            np.ascontiguousarray(
                op[half * NH:(half + 1) * NH].transpose(1, 0, 2).reshape(128, NH * D)),
        ))
    qsc = ((1.0 + qns) / H).reshape(128, 1).astype(np.float32)
    ksc = (1.0 + kns).reshape(128, 1).astype(np.float32)
    ts = THETA ** (np.arange(64, dtype=np.float64) * 2.0 / H)
    fidx = np.arange(QCH)[None, :]
    pidx = np.arange(128)[:, None]
    cm = np.concatenate(
        [(fidx >= m * 128 + pidx).astype(np.float32) for m in range(4)], axis=1)
    cm = np.ascontiguousarray(cm)

    in_maps = []
    for c in range(8):
        b, half = c // 2, c % 2
        wq, wk, wv, wo = halves[half]
        ang = position[b].astype(np.float64)[None, :] / ts[:, None]
        sn = np.sin(ang).astype(np.float32)
        cs = np.cos(ang).astype(np.float32)
        in_maps.append({
            "xt": np.ascontiguousarray(x[b].T),
            "wq": wq, "wk": wk, "wv": wv, "wo": wo,
            "cs": np.ascontiguousarray(np.concatenate([cs, cs], 0)),
            "sn": np.ascontiguousarray(np.concatenate([-sn, sn], 0)),
            "qsc": qsc, "ksc": ksc, "cm": cm,
            "ident": np.eye(128, dtype=np.float32),
            "onesc": np.ones((128, 1), np.float32),
        })

    if TRACE:
        _install_hook()
    last_err = None
    for _ in range(3):
        try:
            res = bass_utils.run_bass_kernel_spmd(nc, in_maps, list(range(8)), trace=TRACE)
            break
        except Exception as e:  # transient NRT device wedge
            last_err = e
    else:
        raise last_err
    LAST_EXEC_NS = getattr(res, "exec_time_ns", None)

    out = np.empty((B, T, D), np.float32)
    for b in range(B):
        out[b] = res.results[2 * b]["out"] + res.results[2 * b + 1]["out"]
    return out



# revision 11
# speedup vs baseline: 1.3552x; 1.3552x over previous
"""MultiHeadAttention (B=4,T=2048,D=2048,NQ=16,NK=8,H=128) on 8 trn2 cores.

Sharding: core c -> batch b=c//2, half=c%2. Each core computes the partial
output for batch b restricted to q-heads [half*8, half*8+8) (kv-heads
[half*4, half*4+4)); host sums the two partials per batch (o_proj
contraction over heads is split across the core pair).

v2: bf16 matmul/vector datapath (PSUM accumulation stays f32), fused
q/k/v projection pass with SBUF-resident q (no DRAM spill), DMA-engine
transposes for V, batched RMSNorm row statistics, f32r broadcast
matmuls, causal-restricted score/exp/PV tiles, software-pipelined
exp/PV attention loop with o_proj fused per 512-row query chunk.
"""
import numpy as np
import concourse.bass as bass
import concourse.tile as tile
from concourse import bacc, mybir
from concourse import bass_utils

B, T, D = 4, 2048, 2048
NQ, NK, H = 16, 8, 128
NH, NKV = 8, 4          # per-core q heads / kv heads
THETA = 10000.0
EPS = 1e-6
TCH = 512               # chunk of T for projections / attention q blocks
NCH = T // TCH
NDK = D // 128
NQC = T // TCH

f32 = mybir.dt.float32
f32r = mybir.dt.float32r
bf16 = mybir.dt.bfloat16
npbf16 = mybir.dt.np(bf16)
AF = mybir.ActivationFunctionType
MUL = mybir.AluOpType.mult

TRACE = False
LAST_EXEC_NS = None
_CACHE = {}


def _install_hook():
    import contextlib, ctypes, sys, types
    if "antenv.axon_hooks" in sys.modules:
        return
    lib = ctypes.CDLL("/opt/axon/libaxon_pjrt.so")
    lib.axon_start_nrt_profile.argtypes = [ctypes.POINTER(ctypes.c_int64), ctypes.c_size_t]
    lib.axon_start_nrt_profile.restype = ctypes.c_int64
    lib.axon_stop_nrt_profile.argtypes = [ctypes.c_char_p]
    lib.axon_stop_nrt_profile.restype = ctypes.c_int64

    @contextlib.contextmanager
    def _hook(output_dir, device_ids):
        import jax
        jax.devices()
        ids = (ctypes.c_int64 * len(device_ids))(*device_ids) if device_ids else None
        rc = lib.axon_start_nrt_profile(ids, len(device_ids) if device_ids else 0)
        if rc != 0:
            raise RuntimeError(f"axon_start_nrt_profile rc={rc}")
        try:
            yield
        finally:
            n = lib.axon_stop_nrt_profile(str(output_dir).encode())
            if n < 0:
                raise RuntimeError(f"axon_stop_nrt_profile rc={n}")

    mod = types.ModuleType("antenv.axon_hooks")
    mod.get_axon_ntff_profile_hook = lambda: _hook
    mod.set_axon_ntff_profile_hook = lambda h: None
    sys.modules["antenv.axon_hooks"] = mod
    bass_utils.upload_artifacts = lambda tmpdir: "local://" + str(tmpdir)


def _build():
    nc = bacc.Bacc("TRN2", target_bir_lowering=False, debug=False, num_devices=8)
    xt_ap = nc.dram_tensor("xt", [D, T], bf16, kind="ExternalInput").ap()
    wq_ap = nc.dram_tensor("wq", [128, NH * NDK * 128], bf16, kind="ExternalInput").ap()
    wk_ap = nc.dram_tensor("wk", [128, NKV * NDK * 128], bf16, kind="ExternalInput").ap()
    wv_ap = nc.dram_tensor("wv", [128, NKV * NDK * 128], bf16, kind="ExternalInput").ap()
    wo_ap = nc.dram_tensor("wo", [128, NH * D], bf16, kind="ExternalInput").ap()
    cs_ap = nc.dram_tensor("cs", [128, T], bf16, kind="ExternalInput").ap()
    sn_ap = nc.dram_tensor("sn", [128, T], bf16, kind="ExternalInput").ap()
    qsc_ap = nc.dram_tensor("qsc", [128, 1], f32, kind="ExternalInput").ap()
    ksc_ap = nc.dram_tensor("ksc", [128, 1], f32, kind="ExternalInput").ap()
    cm_ap = nc.dram_tensor("cm", [128, 128], bf16, kind="ExternalInput").ap()
    out_ap = nc.dram_tensor("out", [T, D], bf16, kind="ExternalOutput").ap()

    with tile.TileContext(nc) as tc:
        with tc.tile_pool(name="mp", bufs=1) as mp, \
             tc.tile_pool(name="pp", bufs=1, space="PSUM") as pp:
            # ---- persistent tiles ----
            cs_t = mp.tile([128, T], bf16)
            nc.sync.dma_start(cs_t[:], cs_ap[:])
            sn_t = mp.tile([128, T], bf16)
            nc.sync.dma_start(sn_t[:], sn_ap[:])
            qsc_t = mp.tile([128, 1], f32)
            nc.sync.dma_start(qsc_t[:], qsc_ap[:])
            ksc_t = mp.tile([128, 1], f32)
            nc.sync.dma_start(ksc_t[:], ksc_ap[:])
            cm_t = mp.tile([128, 128], bf16)
            nc.sync.dma_start(cm_t[:], cm_ap[:])
            eps_t = mp.tile([1, 1], f32)
            nc.vector.memset(eps_t[:], EPS)
            ones_col_b = mp.tile([128, 1], bf16)
            nc.vector.memset(ones_col_b[:], 1.0)
            ones_row_b = mp.tile([1, 128], bf16)
            nc.vector.memset(ones_row_b[:], 1.0)

            qT = mp.tile([128, NH * T], bf16)     # 32KB/part
            kT = mp.tile([128, NKV * T], bf16)    # 16KB
            vT = mp.tile([128, NKV * T], bf16)    # 16KB

            # PSUM bank for softmax denominator rows (attention phase).
            rows = pp.tile([128, 512], f32, tag="rows")

            def drain_group(pool, accs, sc_t, t0, dsts):
                """accs: psum [128,512] f32 -> RMSNorm*(gain) + RoPE -> dsts bf16."""
                n = len(accs)
                for i in range(n):
                    sq = pool.tile([128, TCH], bf16, tag="sq", bufs=2, name=f"sq{i}")
                    nc.scalar.activation(sq[:], accs[i], AF.Square)
                    row = pp.tile([1, 512], f32, tag=f"a{3 + i % 2}", name=f"row{i}")
                    nc.tensor.matmul(row[:], ones_col_b[:], sq[:], start=True, stop=True)
                    lr = pool.tile([1, TCH], f32, tag="lr", bufs=2, name=f"lr{i}")
                    nc.scalar.activation(lr[:], row[:], AF.Ln, bias=eps_t[:], scale=1.0 / H)
                    rstd = pool.tile([1, TCH], bf16, tag="rstd", bufs=2, name=f"rstd{i}")
                    nc.scalar.activation(rstd[:], lr[:], AF.Exp, scale=-0.5)
                    bc = pp.tile([128, 512], f32, tag=f"a{3 + (i + 1) % 2}", name=f"bc{i}")
                    nc.tensor.matmul(bc[:], ones_row_b[:], rstd[:],
                                     start=True, stop=True)
                    # qn0 = acc * gain (per-partition); rstd applied after rope
                    # (valid: rstd is per-column, invariant under the half-swap)
                    qn = pool.tile([128, TCH], bf16, tag="qn", bufs=2, name=f"qn{i}")
                    nc.scalar.activation(qn[:], accs[i], AF.Copy, scale=sc_t[:])
                    qsw = pool.tile([128, TCH], bf16, tag="qsw", bufs=2, name=f"qsw{i}")
                    nc.sync.dma_start(qsw[0:64, :], qn[64:128, :])
                    nc.sync.dma_start(qsw[64:128, :], qn[0:64, :])
                    ta = pool.tile([128, TCH], bf16, tag="ta", bufs=2, name=f"ta{i}")
                    nc.vector.tensor_mul(ta[:], qn[:], cs_t[:, t0:t0 + TCH])
                    tb = pool.tile([128, TCH], bf16, tag="tb", bufs=2, name=f"tb{i}")
                    nc.vector.tensor_mul(tb[:], qsw[:], sn_t[:, t0:t0 + TCH])
                    rs = pool.tile([128, TCH], bf16, tag="rs", bufs=2, name=f"rs{i}")
                    nc.vector.tensor_add(rs[:], ta[:], tb[:])
                    nc.vector.tensor_mul(dsts[i], rs[:], bc[:])

            # ---- phase 1: q/k/v projection, norm+rope, all SBUF-resident ----
            with tc.tile_pool(name="pj", bufs=1) as pj:
                wq_t = pj.tile([128, NH * NDK * 128], bf16)
                for j in range(4):
                    s = NH * NDK * 128 // 4
                    nc.sync.dma_start(wq_t[:, j * s:(j + 1) * s], wq_ap[:, j * s:(j + 1) * s])
                wk_t = pj.tile([128, NKV * NDK * 128], bf16)
                for j in range(2):
                    s = NKV * NDK * 128 // 2
                    nc.sync.dma_start(wk_t[:, j * s:(j + 1) * s], wk_ap[:, j * s:(j + 1) * s])
                wv_t = pj.tile([128, NKV * NDK * 128], bf16)
                for j in range(2):
                    s = NKV * NDK * 128 // 2
                    nc.sync.dma_start(wv_t[:, j * s:(j + 1) * s], wv_ap[:, j * s:(j + 1) * s])

                acc_roll = [0]

                def chain(w_t, head, xh0, xh1):
                    """Sequential 16-dk matmul chain into one rolling psum bank."""
                    acc = pp.tile([128, 512], f32, tag=f"a{acc_roll[0] % 3}", name="acc")
                    acc_roll[0] += 1
                    for dk in range(NDK):
                        xh = xh0 if dk < 8 else xh1
                        nc.tensor.matmul(
                            acc[:],
                            w_t[:, (head * NDK + dk) * 128:(head * NDK + dk + 1) * 128],
                            xh[:, (dk % 8) * TCH:(dk % 8 + 1) * TCH],
                            start=(dk == 0), stop=(dk == NDK - 1))
                    return acc

                for ch in range(NCH):
                    t0 = ch * TCH
                    # x chunk, in two dk-halves for SBUF thrift
                    xh0 = pj.tile([128, 8 * TCH], bf16, tag="xh0", bufs=2, name="xh0")
                    for dk in range(8):
                        nc.sync.dma_start(xh0[:, dk * TCH:(dk + 1) * TCH],
                                          xt_ap[dk * 128:(dk + 1) * 128, t0:t0 + TCH])
                    xh1 = pj.tile([128, 8 * TCH], bf16, tag="xh1", bufs=2, name="xh1")
                    for dk in range(8):
                        nc.sync.dma_start(xh1[:, dk * TCH:(dk + 1) * TCH],
                                          xt_ap[(8 + dk) * 128:(9 + dk) * 128, t0:t0 + TCH])
                    # q heads, two groups of 4
                    for g in range(2):
                        accs = [chain(wq_t, g * 4 + i, xh0, xh1) for i in range(4)]
                        drain_group(pj, [a[:] for a in accs], qsc_t, t0,
                                    [qT[:, (g * 4 + i) * T + t0:(g * 4 + i) * T + t0 + TCH]
                                     for i in range(4)])
                    # k heads
                    accs = [chain(wk_t, i, xh0, xh1) for i in range(NKV)]
                    drain_group(pj, [a[:] for a in accs], ksc_t, t0,
                                [kT[:, kv * T + t0:kv * T + t0 + TCH] for kv in range(NKV)])
                    # v heads: no norm/rope; transpose into [t,H] blocks via DMA xbar
                    for kv in range(NKV):
                        acc = chain(wv_t, kv, xh0, xh1)
                        vtmp = pj.tile([128, TCH], bf16, tag="vtmp", bufs=2, name="vtmp")
                        nc.scalar.activation(vtmp[:], acc[:], AF.Copy)
                        for j in range(TCH // 128):
                            nc.sync.dma_start(
                                vT[:, kv * T + t0 + j * 128:kv * T + t0 + (j + 1) * 128],
                                vtmp[:, j * 128:(j + 1) * 128], transpose=True)

            # ---- phase 2: attention + fused o_proj ----
            with tc.tile_pool(name="op", bufs=1) as op:
                wo_t = op.tile([128, NH * D], bf16)
                for j in range(4):
                    s = NH * D // 4
                    nc.sync.dma_start(wo_t[:, j * s:(j + 1) * s], wo_ap[:, j * s:(j + 1) * s])

                for qi in range(NQC):
                    q0 = qi * TCH
                    attn_sb = op.tile([128, NH * TCH], bf16, tag="attn", bufs=2, name="attn")
                    nkj = 4 * qi + 4
                    for h in range(NH):
                        kv = h // 2
                        o_ps = pp.tile([128, 512], f32, tag=f"a{5 + h % 2}", name="ops")
                        acc_sb = op.tile([128, TCH], bf16, tag="accsb", bufs=2, name="accsb")

                        def emit_s(kj):
                            m = kj - 4 * qi
                            lo = 128 * m if m > 0 else 0
                            s_ps = pp.tile([128, 512], f32, tag=f"a{(0, 3, 4)[kj % 3]}", name="sps")
                            nc.tensor.matmul(
                                s_ps[:, lo:512],
                                kT[:, kv * T + kj * 128:kv * T + (kj + 1) * 128],
                                qT[:, h * T + q0 + lo:h * T + q0 + TCH],
                                start=True, stop=True)
                            pt = op.tile([128, TCH], bf16, tag="pt", bufs=3, name="pt")
                            return s_ps, pt, lo, m, kj

                        def emit_drain(s_ps, pt, lo, m, kj):
                            nc.scalar.activation(pt[:, lo:512], s_ps[:, lo:512], AF.Exp)
                            if m >= 0:
                                nc.vector.tensor_mul(pt[:, lo:lo + 128],
                                                     pt[:, lo:lo + 128], cm_t[:])
                            if kj == 0:
                                nc.vector.tensor_copy(acc_sb[:], pt[:])
                            else:
                                nc.vector.tensor_add(acc_sb[:, lo:512],
                                                     acc_sb[:, lo:512], pt[:, lo:512])
                            nc.tensor.matmul(
                                o_ps[:, lo:512],
                                vT[:, kv * T + kj * 128:kv * T + (kj + 1) * 128],
                                pt[:, lo:512],
                                start=(kj == 0), stop=(kj == nkj - 1),
                                skip_group_check=True)

                        prev = None
                        for kj in range(nkj):
                            cur = emit_s(kj)
                            if prev is not None:
                                emit_drain(*prev)
                            prev = cur
                        emit_drain(*prev)

                        # softmax denominator for this (h, qi)
                        rp = 32 * (h % 3)
                        nc.tensor.matmul(rows[rp:rp + 1, :], ones_col_b[:],
                                         acc_sb[:], start=True, stop=True)
                        rden = op.tile([1, TCH], bf16, tag="rden", bufs=2, name="rden")
                        with nc.allow_low_precision(reason="softmax 1/den in bf16"):
                            nc.vector.reciprocal(rden[:], rows[rp:rp + 1, :])
                        rbc = op.tile([128, TCH], bf16, tag="rbc", bufs=2, name="rbc")
                        nc.gpsimd.partition_broadcast(rbc[:], rden[:])
                        nc.vector.tensor_mul(
                            attn_sb[:, h * TCH:(h + 1) * TCH], o_ps[:], rbc[:])

                    # fused o_proj for this 512-row query chunk
                    for dc in range(4):
                        for ti in range(4):
                            ops2 = pp.tile([128, 512], f32,
                                           tag=f"a{5 + (dc * 4 + ti) % 2}", name="ops2")
                            for h in range(NH):
                                nc.tensor.matmul(
                                    ops2[:],
                                    attn_sb[:, h * TCH + ti * 128:h * TCH + (ti + 1) * 128],
                                    wo_t[:, h * D + dc * 512:h * D + (dc + 1) * 512],
                                    start=(h == 0), stop=(h == NH - 1))
                            stg = op.tile([128, 512], bf16, tag="ostg", bufs=3, name="ostg")
                            if (dc * 4 + ti) % 2 == 0:
                                nc.vector.tensor_copy(stg[:], ops2[:])
                            else:
                                nc.scalar.activation(stg[:], ops2[:], AF.Copy)
                            nc.sync.dma_start(
                                out_ap[q0 + ti * 128:q0 + (ti + 1) * 128,
                                       dc * 512:(dc + 1) * 512], stg[:])

    nc.compile()
    return nc


def _pack(w):
    """(nh, D, H) -> (128, nh*NDK*128): col block (h*NDK+dk)*128 = w[h, dk*128:+128, :]."""
    nh = w.shape[0]
    a = w.reshape(nh, NDK, 128, H).transpose(2, 0, 1, 3)
    return np.ascontiguousarray(a.reshape(128, nh * NDK * H)).astype(npbf16)


def _numpy_ref(x, mask, position, qp, kvp, op, qns, kns):
    def rms(v, s):
        var = (v * v).mean(-1, keepdims=True)
        return v / np.sqrt(var + EPS) * (1.0 + s)

    def rope(v, pos):
        ts = THETA ** (np.arange(64, dtype=np.float32) * 2.0 / H)
        ang = pos.astype(np.float32)[:, :, None, None] / ts
        sn, cs = np.sin(ang), np.cos(ang)
        x1, x2 = v[..., :64], v[..., 64:]
        return np.concatenate([x1 * cs - x2 * sn, x2 * cs + x1 * sn], -1)

    q = np.einsum('BTD,NDH->BTNH', x, qp)
    k = np.einsum('BTD,KDH->BTKH', x, kvp[0])
    v = np.einsum('BTD,KDH->BTKH', x, kvp[1])
    q = rope(rms(q, qns), position) * (H ** -0.5)
    k = rope(rms(k, kns), position)
    q = q.transpose(0, 2, 1, 3)
    k = np.repeat(k.transpose(0, 2, 1, 3), NQ // NK, 1)
    v = np.repeat(v.transpose(0, 2, 1, 3), NQ // NK, 1)
    s = np.einsum('BHtD,BHTD->BHtT', q, k) / np.sqrt(np.float32(H))
    s = np.where(mask[:, None], s, np.float32(-2.3819763e+38))
    s = s - s.max(-1, keepdims=True)
    w = np.exp(s)
    w /= w.sum(-1, keepdims=True)
    o = np.einsum('BHtT,BHTD->BHtD', w, v)
    return np.einsum('BNTH,NHD->BTD', o, op).astype(np.float32)


def kernel(**inputs):
    global LAST_EXEC_NS
    x = np.asarray(inputs["x"], np.float32)
    mask = np.asarray(inputs["mask"])
    position = np.asarray(inputs["position"])
    qp = np.asarray(inputs["q_proj"], np.float32)
    kvp = np.asarray(inputs["kv_proj"], np.float32)
    op = np.asarray(inputs["o_proj"], np.float32)
    qns = np.asarray(inputs["q_norm_scale"], np.float32)
    kns = np.asarray(inputs["k_norm_scale"], np.float32)

    tril = np.tril(np.ones((T, T), bool))
    if mask.shape != (B, T, T) or not all(np.array_equal(mask[b], tril) for b in range(B)):
        return _numpy_ref(x, mask, position, qp, kvp, op, qns, kns)

    if "nc" not in _CACHE:
        _CACHE["nc"] = _build()
    nc = _CACHE["nc"]

    halves = []
    for half in range(2):
        halves.append((
            _pack(qp[half * NH:(half + 1) * NH]),
            _pack(kvp[0, half * NKV:(half + 1) * NKV]),
            _pack(kvp[1, half * NKV:(half + 1) * NKV]),
            np.ascontiguousarray(
                op[half * NH:(half + 1) * NH].transpose(1, 0, 2).reshape(128, NH * D)
            ).astype(npbf16),
        ))
    qsc = ((1.0 + qns) / H).reshape(128, 1).astype(np.float32)
    ksc = (1.0 + kns).reshape(128, 1).astype(np.float32)
    ts = THETA ** (np.arange(64, dtype=np.float64) * 2.0 / H)
    pidx = np.arange(128)[:, None]
    fidx = np.arange(128)[None, :]
    cm = (fidx >= pidx).astype(npbf16)

    in_maps = []
    for c in range(8):
        b, half = c // 2, c % 2
        wq, wk, wv, wo = halves[half]
        ang = position[b].astype(np.float64)[None, :] / ts[:, None]
        sn = np.sin(ang).astype(np.float32)
        cs = np.cos(ang).astype(np.float32)
        in_maps.append({
            "xt": np.ascontiguousarray(x[b].T).astype(npbf16),
            "wq": wq, "wk": wk, "wv": wv, "wo": wo,
            "cs": np.ascontiguousarray(np.concatenate([cs, cs], 0)).astype(npbf16),
            "sn": np.ascontiguousarray(np.concatenate([-sn, sn], 0)).astype(npbf16),
            "qsc": qsc, "ksc": ksc, "cm": cm,
        })

    if TRACE:
        _install_hook()
    last_err = None
    for _ in range(3):
        try:
            res = bass_utils.run_bass_kernel_spmd(nc, in_maps, list(range(8)), trace=TRACE)
            break
        except Exception as e:  # transient NRT device wedge
            last_err = e
    else:
        raise last_err
    LAST_EXEC_NS = getattr(res, "exec_time_ns", None)

    out = np.empty((B, T, D), np.float32)
    for b in range(B):
        out[b] = (res.results[2 * b]["out"].astype(np.float32)
                  + res.results[2 * b + 1]["out"].astype(np.float32))
    return out


# revision 12
# speedup vs baseline: 1.6280x; 1.2013x over previous
"""MultiHeadAttention (B=4,T=2048,D=2048,NQ=16,NK=8,H=128) on 8 trn2 cores.

Sharding: core c -> batch b=c//2, half=c%2. Each core computes the partial
output for batch b restricted to q-heads [half*8, half*8+8) (kv-heads
[half*4, half*4+4)); host sums the two partials per batch (o_proj
contraction over heads is split across the core pair).

v2: bf16 matmul/vector datapath (PSUM accumulation stays f32), fused
q/k/v projection pass with SBUF-resident q (no DRAM spill), DMA-engine
transposes for V, batched RMSNorm row statistics, f32r broadcast
matmuls, causal-restricted score/exp/PV tiles, software-pipelined
exp/PV attention loop with o_proj fused per 512-row query chunk.
"""
import numpy as np
import concourse.bass as bass
import concourse.tile as tile
from concourse import bacc, mybir
from concourse import bass_utils

B, T, D = 4, 2048, 2048
NQ, NK, H = 16, 8, 128
NH, NKV = 8, 4          # per-core q heads / kv heads
THETA = 10000.0
EPS = 1e-6
TCH = 512               # chunk of T for projections / attention q blocks
NCH = T // TCH
NDK = D // 128
NQC = T // TCH

f32 = mybir.dt.float32
f32r = mybir.dt.float32r
bf16 = mybir.dt.bfloat16
npbf16 = mybir.dt.np(bf16)
AF = mybir.ActivationFunctionType
MUL = mybir.AluOpType.mult

TRACE = False
LAST_EXEC_NS = None
_CACHE = {}


def _install_hook():
    import contextlib, ctypes, sys, types
    if "antenv.axon_hooks" in sys.modules:
        return
    lib = ctypes.CDLL("/opt/axon/libaxon_pjrt.so")
    lib.axon_start_nrt_profile.argtypes = [ctypes.POINTER(ctypes.c_int64), ctypes.c_size_t]
    lib.axon_start_nrt_profile.restype = ctypes.c_int64
    lib.axon_stop_nrt_profile.argtypes = [ctypes.c_char_p]
    lib.axon_stop_nrt_profile.restype = ctypes.c_int64

    @contextlib.contextmanager
    def _hook(output_dir, device_ids):
        import jax
        jax.devices()
        ids = (ctypes.c_int64 * len(device_ids))(*device_ids) if device_ids else None
        rc = lib.axon_start_nrt_profile(ids, len(device_ids) if device_ids else 0)
        if rc != 0:
            raise RuntimeError(f"axon_start_nrt_profile rc={rc}")
        try:
            yield
        finally:
            n = lib.axon_stop_nrt_profile(str(output_dir).encode())
            if n < 0:
                raise RuntimeError(f"axon_stop_nrt_profile rc={n}")

    mod = types.ModuleType("antenv.axon_hooks")
    mod.get_axon_ntff_profile_hook = lambda: _hook
    mod.set_axon_ntff_profile_hook = lambda h: None
    sys.modules["antenv.axon_hooks"] = mod
    bass_utils.upload_artifacts = lambda tmpdir: "local://" + str(tmpdir)


def _build():
    nc = bacc.Bacc("TRN2", target_bir_lowering=False, debug=False, num_devices=8)
    xt_ap = nc.dram_tensor("xt", [D, T], bf16, kind="ExternalInput").ap()
    wq_ap = nc.dram_tensor("wq", [128, NH * NDK * 128], bf16, kind="ExternalInput").ap()
    wk_ap = nc.dram_tensor("wk", [128, NKV * NDK * 128], bf16, kind="ExternalInput").ap()
    wv_ap = nc.dram_tensor("wv", [128, NKV * NDK * 128], bf16, kind="ExternalInput").ap()
    wo_ap = nc.dram_tensor("wo", [128, NH * D], bf16, kind="ExternalInput").ap()
    cs_ap = nc.dram_tensor("cs", [128, T], bf16, kind="ExternalInput").ap()
    sn_ap = nc.dram_tensor("sn", [128, T], bf16, kind="ExternalInput").ap()
    qsc_ap = nc.dram_tensor("qsc", [128, 1], f32, kind="ExternalInput").ap()
    ksc_ap = nc.dram_tensor("ksc", [128, 1], f32, kind="ExternalInput").ap()
    cm_ap = nc.dram_tensor("cm", [128, 128], bf16, kind="ExternalInput").ap()
    out_ap = nc.dram_tensor("out", [T, D], bf16, kind="ExternalOutput").ap()

    with tile.TileContext(nc) as tc:
        with tc.tile_pool(name="mp", bufs=1) as mp, \
             tc.tile_pool(name="pp", bufs=1, space="PSUM") as pp:
            # ---- persistent tiles ----
            cs_t = mp.tile([128, T], bf16)
            nc.sync.dma_start(cs_t[:], cs_ap[:])
            sn_t = mp.tile([128, T], bf16)
            nc.sync.dma_start(sn_t[:], sn_ap[:])
            qsc_t = mp.tile([128, 1], f32)
            nc.sync.dma_start(qsc_t[:], qsc_ap[:])
            ksc_t = mp.tile([128, 1], f32)
            nc.sync.dma_start(ksc_t[:], ksc_ap[:])
            cm_t = mp.tile([128, 128], bf16)
            nc.sync.dma_start(cm_t[:], cm_ap[:])
            eps_t = mp.tile([1, 1], f32)
            nc.vector.memset(eps_t[:], EPS)
            ones_col_b = mp.tile([128, 1], bf16)
            nc.vector.memset(ones_col_b[:], 1.0)
            ones_row_b = mp.tile([1, 128], bf16)
            nc.vector.memset(ones_row_b[:], 1.0)

            qT = mp.tile([128, NH * T], bf16)     # 32KB/part
            kT = mp.tile([128, NKV * T], bf16)    # 16KB
            vT = mp.tile([128, NKV * T], bf16)    # 16KB

            # PSUM bank for softmax denominator rows (attention phase).
            rows = pp.tile([128, 512], f32, tag="rows")

            def drain_group(pool, accs, sc_t, t0, dsts):
                """accs: psum [128,512] f32 -> RMSNorm*(gain) + RoPE -> dsts bf16."""
                n = len(accs)
                for i in range(n):
                    sq = pool.tile([128, TCH], bf16, tag="sq", bufs=2, name=f"sq{i}")
                    nc.scalar.activation(sq[:], accs[i], AF.Square)
                    row = pp.tile([1, 512], f32, tag=f"a{3 + i % 2}", name=f"row{i}")
                    nc.tensor.matmul(row[:], ones_col_b[:], sq[:], start=True, stop=True)
                    rinv = pool.tile([1, TCH], f32, tag="rinv", bufs=2, name=f"rinv{i}")
                    nc.vector.reciprocal_approx_fast(rinv[:], row[:])
                    rstd = pool.tile([1, TCH], bf16, tag="rstd", bufs=2, name=f"rstd{i}")
                    nc.scalar.activation(rstd[:], rinv[:], AF.Sqrt)
                    bc = pp.tile([128, 512], f32, tag=f"a{3 + (i + 1) % 2}", name=f"bc{i}")
                    nc.tensor.matmul(bc[:], ones_row_b[:], rstd[:],
                                     start=True, stop=True)
                    # qn0 = acc * gain (per-partition); rstd applied after rope
                    # (valid: rstd is per-column, invariant under the half-swap)
                    qn = pool.tile([128, TCH], bf16, tag="qn", bufs=2, name=f"qn{i}")
                    nc.scalar.activation(qn[:], accs[i], AF.Copy, scale=sc_t[:])
                    qsw = pool.tile([128, TCH], bf16, tag="qsw", bufs=2, name=f"qsw{i}")
                    nc.sync.dma_start(qsw[0:64, :], qn[64:128, :])
                    nc.sync.dma_start(qsw[64:128, :], qn[0:64, :])
                    ta = pool.tile([128, TCH], bf16, tag="ta", bufs=2, name=f"ta{i}")
                    nc.vector.tensor_mul(ta[:], qn[:], cs_t[:, t0:t0 + TCH])
                    tb = pool.tile([128, TCH], bf16, tag="tb", bufs=2, name=f"tb{i}")
                    nc.vector.tensor_mul(tb[:], qsw[:], sn_t[:, t0:t0 + TCH])
                    rs = pool.tile([128, TCH], bf16, tag="rs", bufs=2, name=f"rs{i}")
                    nc.vector.tensor_add(rs[:], ta[:], tb[:])
                    nc.vector.tensor_mul(dsts[i], rs[:], bc[:])

            # ---- phase 1: q/k/v projection, norm+rope, all SBUF-resident ----
            with tc.tile_pool(name="pj", bufs=1) as pj:
                wq_t = pj.tile([128, NH * NDK * 128], bf16)
                for j in range(4):
                    s = NH * NDK * 128 // 4
                    nc.sync.dma_start(wq_t[:, j * s:(j + 1) * s], wq_ap[:, j * s:(j + 1) * s])
                wk_t = pj.tile([128, NKV * NDK * 128], bf16)
                for j in range(2):
                    s = NKV * NDK * 128 // 2
                    nc.sync.dma_start(wk_t[:, j * s:(j + 1) * s], wk_ap[:, j * s:(j + 1) * s])
                wv_t = pj.tile([128, NKV * NDK * 128], bf16)
                for j in range(2):
                    s = NKV * NDK * 128 // 2
                    nc.sync.dma_start(wv_t[:, j * s:(j + 1) * s], wv_ap[:, j * s:(j + 1) * s])

                acc_roll = [0]

                def chain(w_t, head, xh0, xh1):
                    """Sequential 16-dk matmul chain into one rolling psum bank."""
                    acc = pp.tile([128, 512], f32, tag=f"a{acc_roll[0] % 3}", name="acc")
                    acc_roll[0] += 1
                    for dk in range(NDK):
                        xh = xh0 if dk < 8 else xh1
                        nc.tensor.matmul(
                            acc[:],
                            w_t[:, (head * NDK + dk) * 128:(head * NDK + dk + 1) * 128],
                            xh[:, (dk % 8) * TCH:(dk % 8 + 1) * TCH],
                            start=(dk == 0), stop=(dk == NDK - 1))
                    return acc

                for ch in range(NCH):
                    t0 = ch * TCH
                    # x chunk, in two dk-halves for SBUF thrift
                    xh0 = pj.tile([128, 8 * TCH], bf16, tag="xh0", bufs=2, name="xh0")
                    for dk in range(8):
                        nc.sync.dma_start(xh0[:, dk * TCH:(dk + 1) * TCH],
                                          xt_ap[dk * 128:(dk + 1) * 128, t0:t0 + TCH])
                    xh1 = pj.tile([128, 8 * TCH], bf16, tag="xh1", bufs=2, name="xh1")
                    for dk in range(8):
                        nc.sync.dma_start(xh1[:, dk * TCH:(dk + 1) * TCH],
                                          xt_ap[(8 + dk) * 128:(9 + dk) * 128, t0:t0 + TCH])
                    # q heads, two groups of 4
                    for g in range(2):
                        accs = [chain(wq_t, g * 4 + i, xh0, xh1) for i in range(4)]
                        drain_group(pj, [a[:] for a in accs], qsc_t, t0,
                                    [qT[:, (g * 4 + i) * T + t0:(g * 4 + i) * T + t0 + TCH]
                                     for i in range(4)])
                    # k heads
                    accs = [chain(wk_t, i, xh0, xh1) for i in range(NKV)]
                    drain_group(pj, [a[:] for a in accs], ksc_t, t0,
                                [kT[:, kv * T + t0:kv * T + t0 + TCH] for kv in range(NKV)])
                    # v heads: no norm/rope; transpose into [t,H] blocks via DMA xbar
                    for kv in range(NKV):
                        acc = chain(wv_t, kv, xh0, xh1)
                        vtmp = pj.tile([128, TCH], bf16, tag="vtmp", bufs=2, name="vtmp")
                        nc.scalar.activation(vtmp[:], acc[:], AF.Copy)
                        for j in range(TCH // 128):
                            nc.sync.dma_start(
                                vT[:, kv * T + t0 + j * 128:kv * T + t0 + (j + 1) * 128],
                                vtmp[:, j * 128:(j + 1) * 128], transpose=True)

            # ---- phase 2: attention + fused o_proj ----
            with tc.tile_pool(name="op", bufs=1) as op:
                wo_t = op.tile([128, NH * D], bf16)
                for j in range(4):
                    s = NH * D // 4
                    nc.sync.dma_start(wo_t[:, j * s:(j + 1) * s], wo_ap[:, j * s:(j + 1) * s])

                for qi in range(NQC):
                    q0 = qi * TCH
                    attn_sb = op.tile([128, NH * TCH], bf16, tag="attn", bufs=2, name="attn")
                    nkj = 4 * qi + 4
                    for h in range(NH):
                        kv = h // 2
                        o_ps = pp.tile([128, 512], f32, tag=f"a{5 + h % 2}", name="ops")
                        acc_sb = op.tile([128, TCH], bf16, tag="accsb", bufs=2, name="accsb")

                        def emit_s(kj):
                            m = kj - 4 * qi
                            lo = 128 * m if m > 0 else 0
                            s_ps = pp.tile([128, 512], f32, tag=f"a{(0, 3, 4)[kj % 3]}", name="sps")
                            nc.tensor.matmul(
                                s_ps[:, lo:512],
                                kT[:, kv * T + kj * 128:kv * T + (kj + 1) * 128],
                                qT[:, h * T + q0 + lo:h * T + q0 + TCH],
                                start=True, stop=True)
                            pt = op.tile([128, TCH], bf16, tag="pt", bufs=3, name="pt")
                            return s_ps, pt, lo, m, kj

                        def emit_drain(s_ps, pt, lo, m, kj):
                            nc.scalar.activation(pt[:, lo:512], s_ps[:, lo:512], AF.Exp)
                            if m >= 0:
                                nc.vector.tensor_mul(pt[:, lo:lo + 128],
                                                     pt[:, lo:lo + 128], cm_t[:])
                            if kj == 0:
                                nc.vector.tensor_copy(acc_sb[:], pt[:])
                            else:
                                nc.vector.tensor_add(acc_sb[:, lo:512],
                                                     acc_sb[:, lo:512], pt[:, lo:512])
                            nc.tensor.matmul(
                                o_ps[:, lo:512],
                                vT[:, kv * T + kj * 128:kv * T + (kj + 1) * 128],
                                pt[:, lo:512],
                                start=(kj == 0), stop=(kj == nkj - 1),
                                skip_group_check=True)

                        prev = None
                        for kj in range(nkj):
                            cur = emit_s(kj)
                            if prev is not None:
                                emit_drain(*prev)
                            prev = cur
                        emit_drain(*prev)

                        # softmax denominator for this (h, qi)
                        rp = 32 * (h % 3)
                        nc.tensor.matmul(rows[rp:rp + 1, :], ones_col_b[:],
                                         acc_sb[:], start=True, stop=True)
                        rden = op.tile([1, TCH], f32, tag="rden", bufs=2, name="rden")
                        nc.vector.reciprocal_approx_fast(rden[:], rows[rp:rp + 1, :])
                        rbc = op.tile([128, TCH], f32, tag="rbc", bufs=2, name="rbc")
                        nc.gpsimd.partition_broadcast(rbc[:], rden[:])
                        nc.vector.tensor_mul(
                            attn_sb[:, h * TCH:(h + 1) * TCH], o_ps[:], rbc[:])

                    # fused o_proj for this 512-row query chunk
                    for dc in range(4):
                        for ti in range(4):
                            ops2 = pp.tile([128, 512], f32,
                                           tag=f"a{5 + (dc * 4 + ti) % 2}", name="ops2")
                            for h in range(NH):
                                nc.tensor.matmul(
                                    ops2[:],
                                    attn_sb[:, h * TCH + ti * 128:h * TCH + (ti + 1) * 128],
                                    wo_t[:, h * D + dc * 512:h * D + (dc + 1) * 512],
                                    start=(h == 0), stop=(h == NH - 1))
                            stg = op.tile([128, 512], bf16, tag="ostg", bufs=3, name="ostg")
                            if (dc * 4 + ti) % 2 == 0:
                                nc.vector.tensor_copy(stg[:], ops2[:])
                            else:
                                nc.scalar.activation(stg[:], ops2[:], AF.Copy)
                            nc.sync.dma_start(
                                out_ap[q0 + ti * 128:q0 + (ti + 1) * 128,
                                       dc * 512:(dc + 1) * 512], stg[:])

    nc.compile()
    return nc


def _pack(w):
    """(nh, D, H) -> (128, nh*NDK*128): col block (h*NDK+dk)*128 = w[h, dk*128:+128, :]."""
    nh = w.shape[0]
    a = w.reshape(nh, NDK, 128, H).transpose(2, 0, 1, 3)
    return np.ascontiguousarray(a.reshape(128, nh * NDK * H)).astype(npbf16)


def _numpy_ref(x, mask, position, qp, kvp, op, qns, kns):
    def rms(v, s):
        var = (v * v).mean(-1, keepdims=True)
        return v / np.sqrt(var + EPS) * (1.0 + s)

    def rope(v, pos):
        ts = THETA ** (np.arange(64, dtype=np.float32) * 2.0 / H)
        ang = pos.astype(np.float32)[:, :, None, None] / ts
        sn, cs = np.sin(ang), np.cos(ang)
        x1, x2 = v[..., :64], v[..., 64:]
        return np.concatenate([x1 * cs - x2 * sn, x2 * cs + x1 * sn], -1)

    q = np.einsum('BTD,NDH->BTNH', x, qp)
    k = np.einsum('BTD,KDH->BTKH', x, kvp[0])
    v = np.einsum('BTD,KDH->BTKH', x, kvp[1])
    q = rope(rms(q, qns), position) * (H ** -0.5)
    k = rope(rms(k, kns), position)
    q = q.transpose(0, 2, 1, 3)
    k = np.repeat(k.transpose(0, 2, 1, 3), NQ // NK, 1)
    v = np.repeat(v.transpose(0, 2, 1, 3), NQ // NK, 1)
    s = np.einsum('BHtD,BHTD->BHtT', q, k) / np.sqrt(np.float32(H))
    s = np.where(mask[:, None], s, np.float32(-2.3819763e+38))
    s = s - s.max(-1, keepdims=True)
    w = np.exp(s)
    w /= w.sum(-1, keepdims=True)
    o = np.einsum('BHtT,BHTD->BHtD', w, v)
    return np.einsum('BNTH,NHD->BTD', o, op).astype(np.float32)


def kernel(**inputs):
    global LAST_EXEC_NS
    x = np.asarray(inputs["x"], np.float32)
    mask = np.asarray(inputs["mask"])
    position = np.asarray(inputs["position"])
    qp = np.asarray(inputs["q_proj"], np.float32)
    kvp = np.asarray(inputs["kv_proj"], np.float32)
    op = np.asarray(inputs["o_proj"], np.float32)
    qns = np.asarray(inputs["q_norm_scale"], np.float32)
    kns = np.asarray(inputs["k_norm_scale"], np.float32)

    tril = np.tril(np.ones((T, T), bool))
    if mask.shape != (B, T, T) or not all(np.array_equal(mask[b], tril) for b in range(B)):
        return _numpy_ref(x, mask, position, qp, kvp, op, qns, kns)

    if "nc" not in _CACHE:
        _CACHE["nc"] = _build()
    nc = _CACHE["nc"]

    halves = []
    for half in range(2):
        halves.append((
            _pack(qp[half * NH:(half + 1) * NH]),
            _pack(kvp[0, half * NKV:(half + 1) * NKV]),
            _pack(kvp[1, half * NKV:(half + 1) * NKV]),
            np.ascontiguousarray(
                op[half * NH:(half + 1) * NH].transpose(1, 0, 2).reshape(128, NH * D)
            ).astype(npbf16),
        ))
    qsc = ((1.0 + qns) / np.sqrt(H)).reshape(128, 1).astype(np.float32)
    ksc = ((1.0 + kns) * np.sqrt(H)).reshape(128, 1).astype(np.float32)
    ts = THETA ** (np.arange(64, dtype=np.float64) * 2.0 / H)
    pidx = np.arange(128)[:, None]
    fidx = np.arange(128)[None, :]
    cm = (fidx >= pidx).astype(npbf16)

    in_maps = []
    for c in range(8):
        b, half = c // 2, c % 2
        wq, wk, wv, wo = halves[half]
        ang = position[b].astype(np.float64)[None, :] / ts[:, None]
        sn = np.sin(ang).astype(np.float32)
        cs = np.cos(ang).astype(np.float32)
        in_maps.append({
            "xt": np.ascontiguousarray(x[b].T).astype(npbf16),
            "wq": wq, "wk": wk, "wv": wv, "wo": wo,
            "cs": np.ascontiguousarray(np.concatenate([cs, cs], 0)).astype(npbf16),
            "sn": np.ascontiguousarray(np.concatenate([-sn, sn], 0)).astype(npbf16),
            "qsc": qsc, "ksc": ksc, "cm": cm,
        })

    if TRACE:
        _install_hook()
    last_err = None
    for _ in range(3):
        try:
            res = bass_utils.run_bass_kernel_spmd(nc, in_maps, list(range(8)), trace=TRACE)
            break
        except Exception as e:  # transient NRT device wedge
            last_err = e
    else:
        raise last_err
    LAST_EXEC_NS = getattr(res, "exec_time_ns", None)

    out = np.empty((B, T, D), np.float32)
    for b in range(B):
        out[b] = (res.results[2 * b]["out"].astype(np.float32)
                  + res.results[2 * b + 1]["out"].astype(np.float32))
    return out
